# revision 22
# baseline (speedup 1.0000x reference)
"""Multi-head attention (B=2, S=2048, D=768, H=12) on 8 NeuronCores.

Sharding: 8 cores = 2 batches x 4 head-groups (3 heads each).
Each core computes q/k/v projections for its head group, causal flash
attention, and a partial output projection over the full 768 output
columns.  The 4 partials per batch are summed on the host (cheap) --
no on-device collective needed.

Device dataflow (per core), all activations kept in transposed
[feature, seq] layout so every DMA is contiguous (host pre-transposes):
  qT/kT/vT = W @ X^T            (PE, f32r)
  scoresT[sk, sq] = kT.T @ qT   (PE, f32r; per 128-wide k-tile)
  expT = exp(scores/8 - 4)      (ACT, fused scale+bias; constant shift
                                 cancels in softmax, guards overflow)
  mask: multiply by 0/1 tile    (DVE, only tiles where mask is mixed)
  ctx_augT[65, sq] += v_aug.T @ expT   (PE; v_aug has a ones column so
                                 row 64 accumulates the softmax denom)
  normalize via reciprocal + PE ones-broadcast, then
  out[sq, 768] = sum_h ctxT_h.T @ Wo_h    (PE, partial over heads)

The mask input is classified numpy-side per (q-tile, k-tile) block into
skip / full / mixed (mixed blocks deduped and shipped as 0/1 tiles), so
any mask pattern works; the causal mask yields 4 unique mixed tiles.
"""

import numpy as np

import concourse.bass as bass
import concourse.tile as tile
from concourse import bacc, mybir
from concourse import bass_utils

# ---- problem constants (hardcoded per contract) ----
B, S, D, H = 2, 2048, 768, 12
N_CORES = 8
GROUPS = 4                 # head-group parallel degree
HPC = H // GROUPS          # heads per core = 3
DK = D // H                # 64
DG = HPC * DK              # 192 dims per group
SQ = 512                   # q tile (free dim) width
SK = 128                   # k tile (partition) width
NQT = S // SQ              # 4
NKT = S // SK              # 16
ND = D // 128              # 6 contraction tiles
SCALE = 1.0 / np.sqrt(DK)  # 0.125
EBIAS = -4.0               # constant shift inside exp

F32 = mybir.dt.float32
F32R = mybir.dt.float32r

_BUILD_CACHE = {}
_last_in_maps = None


def _classify_mask(mask2d):
    """Per (qt, kt) block status for scoresT tiles.

    Returns (status, mask_tiles):
      status[qt][kt] = 'skip' | 'full' | ('mask', idx)
      mask_tiles: float32 array (n, 128, 512) of 0/1 multiplicative masks
                  in scoresT orientation (sk partition, sq free).
    """
    status = [[None] * NKT for _ in range(NQT)]
    uniq = {}
    tiles = []
    for qt in range(NQT):
        qs = qt * SQ
        for kt in range(NKT):
            ks = kt * SK
            blk = (mask2d[qs:qs + SQ, ks:ks + SK] != 0)  # [sq, sk]
            if not blk.any():
                status[qt][kt] = 'skip'
            elif blk.all():
                status[qt][kt] = 'full'
            else:
                t = np.ascontiguousarray(blk.T.astype(np.float32))
                key = t.tobytes()
                if key not in uniq:
                    uniq[key] = len(tiles)
                    tiles.append(t)
                status[qt][kt] = ('mask', uniq[key])
    mask_tiles = (np.stack(tiles) if tiles
                  else np.zeros((0, SK, SQ), np.float32))
    trims = []
    for t in tiles:
        colsum = t.sum(0)
        nz = np.nonzero(colsum)[0]
        lead = int(nz[0]) if len(nz) else 0
        trail = int(SQ - 1 - nz[-1]) if len(nz) else 0
        # keep matmul N >= 256 (f32r full rate) and 4-aligned offsets
        lead = min(lead, 256) & ~3
        trail = min(trail, SQ - 256 - lead) & ~3
        trims.append((lead, max(trail, 0)))
    return status, mask_tiles, trims


def _build(status_key, status, trims, n_mask, repeat=1):
    """Build + compile the SPMD program for a given mask structure."""
    nc = bacc.Bacc("TRN2", target_bir_lowering=False, debug=False,
                   enable_asserts=False, num_devices=N_CORES)

    qt_d = nc.dram_tensor("qt_in", [D, S], F32R, kind="ExternalInput").ap()
    kt_d = nc.dram_tensor("kt_in", [D, S], F32R, kind="ExternalInput").ap()
    vt_d = nc.dram_tensor("vt_in", [D, S], F32R, kind="ExternalInput").ap()
    wqkv_d = nc.dram_tensor("wqkv", [D, 3 * DG], F32R, kind="ExternalInput").ap()
    wo_d = nc.dram_tensor("wo", [DG, D], F32R, kind="ExternalInput").ap()
    ident_d = nc.dram_tensor("ident", [128, 128], F32R, kind="ExternalInput").ap()
    if n_mask:
        cmask_d = nc.dram_tensor("cmask", [n_mask, SK, SQ], F32R,
                                 kind="ExternalInput").ap()
    out_d = nc.dram_tensor("out", [S, D], F32, kind="ExternalOutput").ap()

    # raw constant tiles (initialized before the Tile region)
    bias_t = nc.alloc_sbuf_tensor("c_ebias", [128, 1], F32)
    nc.gpsimd.memset(bias_t.ap(), EBIAS)
    ones_col = nc.alloc_sbuf_tensor("c_ones_col", [128, 1], F32R)
    nc.gpsimd.memset(ones_col.ap().bitcast(F32), 1.0)
    ones_r1 = nc.alloc_sbuf_tensor("c_ones_r1", [1, 64], F32R)
    nc.gpsimd.memset(ones_r1.ap().bitcast(F32), 1.0)
    nc.all_engine_barrier()

    EXP = mybir.ActivationFunctionType.Exp
    CW = 576  # 3*DG, wqkv width
    # weight chunk offsets within a 576 block: (off, width, dest)
    WCHUNKS = [(0, 128, 'q', 0), (128, 64, 'q', 1),
               (192, 128, 'k', 0), (320, 64, 'k', 1),
               (384, 128, 'v', 0), (512, 64, 'v', 1)]

    with tile.TileContext(nc) as tc:
        with tc.tile_pool(name="persist", bufs=1) as pp, \
             tc.tile_pool(name="xin", bufs=3) as xin, \
             tc.tile_pool(name="expp", bufs=3) as expp, \
             tc.tile_pool(name="outp", bufs=4) as outp, \
             tc.tile_pool(name="nrm", bufs=2) as nrm, \
             tc.tile_pool(name="ps", bufs=1, space="PSUM") as psp:

            # ---- resident SBUF tensors ----
            wqkv_sb = pp.tile([128, ND * CW], F32R, tag="wqkv")
            for d in range(ND):
                nc.sync.dma_start(wqkv_sb[:, d * CW:(d + 1) * CW],
                                  wqkv_d[d * 128:(d + 1) * 128, :])
            wo_sb = [pp.tile([64, D], F32R, tag=f"wo{h}", name=f"wo{h}")
                     for h in range(HPC)]
            ident_sb = pp.tile([128, 128], F32R, tag="ident")
            cm_sb = [pp.tile([SK, SQ], F32R, tag=f"cm{i}", name=f"cm{i}")
                     for i in range(n_mask)]

            def load_late_weights():
                nc.sync.dma_start(ident_sb[:], ident_d[:])
                for i in range(n_mask):
                    nc.sync.dma_start(cm_sb[i][:], cmask_d[i])
                for h in range(HPC):
                    nc.sync.dma_start(wo_sb[h][:],
                                      wo_d[h * 64:(h + 1) * 64, :])

            qA = pp.tile([128, S], F32R, tag="qA")   # heads 0,1 (dims 0:128)
            qB = pp.tile([64, S], F32R, tag="qB")    # head 2
            kA = pp.tile([128, S], F32R, tag="kA")
            kB = pp.tile([64, S], F32R, tag="kB")
            vA = pp.tile([128, S], F32R, tag="vA")   # vT, heads 0,1
            vB = pp.tile([64, S], F32R, tag="vB")    # vT, head 2
            v_sb = pp.tile([128, NKT * HPC * 65], F32R, tag="v_sb")
            ctx_stage = pp.tile([65, HPC * NQT * SQ], F32, tag="ctxs")
            ctxn = pp.tile([64, HPC * NQT * SQ], F32R, tag="ctxn")

            # ones column of every v_aug block
            v_blocks = v_sb[:].rearrange("p (t c) -> p t c", c=65)
            nc.vector.tensor_copy(v_blocks[:, :, 64:65],
                                  ones_col.ap().to_broadcast([128, NKT * HPC, 1]))

            # weight sub-chunk layout inside each 576-wide d block:
            # (offset, width, which input, psum tag)
            PAIRS = [('q', [(0, 128, "pa"), (128, 64, "pb")], qt_d, qA, qB),
                     ('k', [(192, 128, "pc"), (320, 64, "pd")], kt_d, kA, kB),
                     ('v', [(384, 128, "pa"), (512, 64, "pb")], vt_d, vA, vB)]

            def trim_of(qt, kt):
                st = status[qt][kt]
                if isinstance(st, tuple):
                    return trims[st[1]]
                return (0, 0)

            import contextlib
            rep_cm = (tc.For_i(0, repeat, 1) if repeat > 1
                      else contextlib.nullcontext())
            with rep_cm:
             for c in range(NQT):
                # ---- projections for s-chunk c ----
                for nm, chunks, x_d, big, small in PAIRS:
                    pst = {}
                    for off, w, tag in chunks:
                        pst[off] = psp.tile([128 if w == 128 else 64, SQ],
                                            F32, tag=tag, name=f"ps{nm}{off}")
                    for d in range(ND):
                        xt = xin.tile([128, SQ], F32R, tag=f"x{nm}",
                                      name=f"x{nm}{c}{d}")
                        nc.sync.dma_start(xt[:], x_d[d * 128:(d + 1) * 128,
                                                     c * SQ:(c + 1) * SQ])
                        for off, w, tag in chunks:
                            nc.tensor.matmul(
                                pst[off][:],
                                wqkv_sb[:, d * CW + off:d * CW + off + w],
                                xt[:], start=(d == 0), stop=(d == ND - 1))
                    for off, w, tag in chunks:
                        dst = big if w == 128 else small
                        nc.vector.tensor_copy(dst[:, c * SQ:(c + 1) * SQ],
                                              pst[off][:])

                if c == 0:
                    load_late_weights()

                # ---- vT -> v natural for k-tiles of this chunk ----
                for t in range(4 * c, 4 * c + 4):
                    tr = psp.tile([128, 192], F32R, tag="pc", name=f"tr_{t}")
                    nc.tensor.transpose(tr[:, 0:128],
                                        vA[:, t * 128:(t + 1) * 128],
                                        ident_sb[:])
                    nc.tensor.transpose(tr[:, 128:192],
                                        vB[:, t * 128:(t + 1) * 128],
                                        ident_sb[0:64, 0:64])
                    base = t * HPC * 65
                    nc.scalar.copy(v_sb[:, base:base + 64], tr[:, 0:64])
                    nc.scalar.copy(v_sb[:, base + 65:base + 129],
                                   tr[:, 64:128])
                    nc.scalar.copy(v_sb[:, base + 130:base + 194],
                                   tr[:, 128:192])

                # ---- attention for qt = c ----
                qt = c
                any_kt = False
                for h in range(HPC):
                    if h < 2:
                        k_lhs = kA[h * 64:(h + 1) * 64, :]
                        q_rhs = qA[h * 64:(h + 1) * 64, :]
                    else:
                        k_lhs = kB[:]
                        q_rhs = qB[:]
                    cidx = qt * HPC + h
                    kts = [kt for kt in range(NKT) if status[qt][kt] != 'skip']
                    if not kts:
                        nc.any.memset(
                            ctxn[:, cidx * SQ:(cidx + 1) * SQ].bitcast(F32),
                            0.0)
                        continue
                    any_kt = True
                    ctx = psp.tile([65, SQ], F32, tag="ctx", name=f"ctx{cidx}")
                    for j, kt in enumerate(kts):
                        lead, trail = trim_of(qt, kt)
                        lo, hi = lead, SQ - trail
                        sc = psp.tile([128, SQ], F32, tag="sc", bufs=2,
                                      name=f"sc{cidx}_{kt}")
                        nc.tensor.matmul(
                            sc[:, lo:hi], k_lhs[:, kt * SK:(kt + 1) * SK],
                            q_rhs[:, qt * SQ + lo:qt * SQ + hi],
                            start=True, stop=True)
                        ex = expp.tile([128, SQ], F32R, tag="ex",
                                       name=f"ex{cidx}_{kt}")
                        nc.scalar.activation(ex[:, lo:hi], sc[:, lo:hi],
                                             EXP, bias=bias_t.ap(),
                                             scale=float(SCALE))
                        st = status[qt][kt]
                        if isinstance(st, tuple):
                            nc.vector.tensor_mul(ex[:, lo:hi], ex[:, lo:hi],
                                                 cm_sb[st[1]][:, lo:hi])
                        vbase = (kt * HPC + h) * 65
                        nc.tensor.matmul(ctx[:, lo:hi],
                                         v_sb[:, vbase:vbase + 65],
                                         ex[:, lo:hi],
                                         start=(j == 0), stop=(j == len(kts) - 1))
                    nc.vector.tensor_copy(
                        ctx_stage[:, cidx * SQ:(cidx + 1) * SQ], ctx[:])

                # ---- normalize qt's heads ----
                if any_kt:
                    den_t = nrm.tile([12, 128], F32, tag="den", name=f"den{qt}")
                    nc.sync.dma_start(
                        den_t[:],
                        ctx_stage[64:65, qt * HPC * SQ:(qt + 1) * HPC * SQ])
                    rec_t = nrm.tile([12, 128], F32, tag="rec", name=f"rec{qt}")
                    nc.vector.reciprocal(rec_t[:], den_t[:])
                    rec_r = nrm.tile([12, 128], F32R, tag="recr",
                                     name=f"recr{qt}")
                    nc.vector.tensor_copy(rec_r[:], rec_t[:])
                    rec_f = nrm.tile([1, HPC * SQ], F32R, tag="recf",
                                     name=f"recf{qt}")
                    nc.sync.dma_start(rec_f[:], rec_r[:])
                    for h in range(HPC):
                        cidx = qt * HPC + h
                        bc = psp.tile([64, SQ], F32, tag="mix",
                                      name=f"bc{cidx}")
                        nc.tensor.matmul(bc[:], ones_r1.ap(),
                                         rec_f[:, h * SQ:(h + 1) * SQ],
                                         start=True, stop=True)
                        nc.vector.tensor_mul(
                            ctxn[:, cidx * SQ:(cidx + 1) * SQ],
                            ctx_stage[0:64, cidx * SQ:(cidx + 1) * SQ],
                            bc[:])

                # ---- output projection for this qt's 4 row-chunks ----
                for sqc in range(4 * qt, 4 * qt + 4):
                    o = (sqc % 4) * 128
                    ot = outp.tile([128, D], F32, tag="ot", name=f"ot{sqc}")
                    for half in range(2):
                        ptag, pbufs = "mix", 1
                        if half == 1 and qt == NQT - 1:
                            ptag, pbufs = "sc", 2
                        po = psp.tile([128, 384], F32, tag=ptag, bufs=pbufs,
                                      name=f"po{sqc}_{half}")
                        for h in range(HPC):
                            cbase = (qt * HPC + h) * SQ
                            nc.tensor.matmul(
                                po[:], ctxn[:, cbase + o:cbase + o + 128],
                                wo_sb[h][:, half * 384:(half + 1) * 384],
                                start=(h == 0), stop=(h == HPC - 1))
                        nc.vector.tensor_copy(ot[:, half * 384:(half + 1) * 384],
                                              po[:])
                        nc.sync.dma_start(
                            out_d[sqc * 128:(sqc + 1) * 128,
                                  half * 384:(half + 1) * 384],
                            ot[:, half * 384:(half + 1) * 384])

    nc.compile()
    return nc


def kernel(Q, K, V, mask, Wq, Wk, Wv, Wo):
    Q = np.asarray(Q, np.float32)
    K = np.asarray(K, np.float32)
    V = np.asarray(V, np.float32)
    Wq = np.asarray(Wq, np.float32)
    Wk = np.asarray(Wk, np.float32)
    Wv = np.asarray(Wv, np.float32)
    Wo = np.asarray(Wo, np.float32)
    mask2d = np.asarray(mask).reshape(S, S)

    status, mask_tiles, trims = _classify_mask(mask2d)
    skey = (tuple(tuple(repr(s) for s in row) for row in status),
            mask_tiles.tobytes())
    ckey = (skey[0], hash(skey[1]))
    if ckey not in _BUILD_CACHE:
        _BUILD_CACHE[ckey] = _build(ckey, status, trims, len(mask_tiles))
    nc = _BUILD_CACHE[ckey]

    ident = np.eye(128, dtype=np.float32)
    QT = [np.ascontiguousarray(Q[b].T) for b in range(B)]
    KT = [np.ascontiguousarray(K[b].T) for b in range(B)]
    VT = [np.ascontiguousarray(V[b].T) for b in range(B)]
    in_maps = []
    for b in range(B):
        for g in range(GROUPS):
            sl = slice(g * DG, (g + 1) * DG)
            wqkv = np.ascontiguousarray(
                np.concatenate([Wq[sl].T, Wk[sl].T, Wv[sl].T], axis=1))
            wo = np.ascontiguousarray(Wo[:, sl].T)
            m = {"qt_in": QT[b], "kt_in": KT[b], "vt_in": VT[b],
                 "wqkv": wqkv, "wo": wo, "ident": ident}
            if len(mask_tiles):
                m["cmask"] = mask_tiles
            in_maps.append(m)

    global _last_in_maps
    _last_in_maps = in_maps
    res = bass_utils.run_bass_kernel_spmd(nc, in_maps,
                                          core_ids=list(range(N_CORES)))
    out = np.zeros((B, S, D), np.float32)
    for b in range(B):
        for g in range(GROUPS):
            out[b] += res.results[b * GROUPS + g]["out"]
    return out
